# revision 16
# baseline (speedup 1.0000x reference)
"""MoE block (top-2 routing, 8 experts) on 8 Trainium2 NeuronCores.

Strategy (expert-parallel, as in the sharding hint):
  - Router (logits -> softmax -> top-2 -> gates) runs on host in f64 as part
    of sharding: the dispatch *is* the shard step, and the gather indices are
    needed on host anyway to build per-core batches and to unshard.
  - Core e owns expert e: it receives the tokens routed to expert e
    (padded to a common capacity C) plus W1[e], W2[e], and computes
    y = gelu(x @ W1) @ W2 * gate with float32r matmuls (full fp32 data,
    ~1 cycle/row on the PE like bf16, ~1e-4 rel err).
  - W1/W2 in fp32 do not fit in SBUF (256 KB/partition), so H=4096 is split
    into two phases of 2048; each phase keeps its W1/W2 halves resident and
    streams token chunks. Each phase writes its partial y (ya / yb).
  - Host combine: out[token] += ya + yb for each expert's token list (each
    token appears in exactly 2 expert lists). Load-balance loss lb is
    computed on host from the routing stats.
"""

import os

import numpy as np

import concourse.bass as bass  # noqa: F401  (engine types come via bacc)
import concourse.mybir as mybir
import concourse.tile as tile
from concourse import bacc
from concourse.bass_utils import run_bass_kernel_spmd

P = 128
D = 1024
H = 4096
E = 8
TOPK = 2
EPS = 1e-9
KD = D // P          # 8 k-tiles over D
KH = H // P          # 32 h-tiles over H
KHP = KH // 2        # 16 h-tiles per phase
F32 = mybir.dt.float32
F32R = mybir.dt.float32r

LAST_EXEC_TIME_NS = None
LAST_RESULTS = None

_NC_CACHE = {}


def _chunks_for(n128):
    """Token-chunk sizes (multiples of 128, each >=256 so float32r matmuls
    run at 1 cycle/row) covering n128*128 tokens."""
    n128 = max(n128, 2)
    full, r = divmod(n128, 4)
    chunks = [512] * full
    if r == 1:
        if full:
            chunks[-1] = 384
            chunks.append(256)
        else:
            chunks = [256]
    elif r == 2:
        chunks.append(256)
    elif r == 3:
        chunks.append(384)
    return chunks


def _build(chunks):
    C = sum(chunks)
    CT = C // P
    nc = bacc.Bacc(None, target_bir_lowering=False)

    xt_d = nc.dram_tensor("xt", [P, KD, C], F32R, kind="ExternalInput")
    w1_d = nc.dram_tensor("w1", [KH, P, KD, P], F32R, kind="ExternalInput")
    w2_d = nc.dram_tensor("w2", [KH, P, D], F32R, kind="ExternalInput")
    g_d = nc.dram_tensor("g", [P, CT], F32, kind="ExternalInput")
    ya_d = nc.dram_tensor("ya", [P, CT, D], F32, kind="ExternalOutput")
    yb_d = nc.dram_tensor("yb", [P, CT, D], F32, kind="ExternalOutput")

    with tile.TileContext(nc) as tc:
        with (
            tc.tile_pool(name="wpool", bufs=1) as wpool,
            tc.tile_pool(name="xpool", bufs=2) as xpool,
            tc.tile_pool(name="hpool", bufs=1) as hpool,
            tc.tile_pool(name="ypool", bufs=4) as ypool,
            tc.tile_pool(name="gpool", bufs=1) as gpool,
            tc.tile_pool(name="ps1", bufs=6, space="PSUM") as ps1,
            tc.tile_pool(name="ps2", bufs=2, space="PSUM") as ps2,
        ):
            g_sb = gpool.tile([P, CT], F32)
            nc.sync.dma_start(g_sb[:], g_d[:])

            for phase, y_d in enumerate((ya_d, yb_d)):
                # chunk-0 x before the 16 MB of weights, so the first
                # matmul isn't queued behind the whole weight load
                xt0 = xpool.tile([P, KD, chunks[0]], F32R, tag="xt")
                nc.sync.dma_start(xt0[:], xt_d[:, :, 0:chunks[0]])

                w1_t = []
                for mh in range(KHP):
                    t = wpool.tile([P, KD, P], F32R, tag=f"w1_{mh}")
                    nc.sync.dma_start(t[:], w1_d[phase * KHP + mh])
                    w1_t.append(t)
                w2_t = []
                for kh in range(KHP):
                    t = wpool.tile([P, D], F32R, tag=f"w2_{kh}")
                    nc.sync.dma_start(t[:], w2_d[phase * KHP + kh])
                    w2_t.append(t)

                t0 = 0
                for ci, tcs in enumerate(chunks):
                    if ci == 0:
                        xt = xt0
                    else:
                        xt = xpool.tile([P, KD, tcs], F32R, tag="xt")
                        nc.sync.dma_start(xt[:], xt_d[:, :, t0:t0 + tcs])

                    # h[mh] = gelu(W1_tile.T @ x_chunk), laid out (H-part, token)
                    h_t = []
                    for mh in range(KHP):
                        ps = ps1.tile([P, tcs], F32, tag="ps")
                        for kd in range(KD):
                            nc.tensor.matmul(
                                ps[:],
                                w1_t[mh][:, kd, :],
                                xt[:, kd, :],
                                start=(kd == 0),
                                stop=(kd == KD - 1),
                            )
                        hb = hpool.tile([P, tcs], F32R, tag=f"h{mh}")
                        nc.scalar.activation(
                            hb[:], ps[:], mybir.ActivationFunctionType.Gelu
                        )
                        h_t.append(hb)

                    # y[token-tile, d] = h.T @ W2, scaled by per-token gate
                    for mt in range(tcs // P):
                        gcol = g_sb[:, t0 // P + mt: t0 // P + mt + 1]
                        for nd in range(2):
                            ys = ps2.tile([P, 512], F32, tag="ys")
                            for kh in range(KHP):
                                nc.tensor.matmul(
                                    ys[:],
                                    h_t[kh][:, mt * P:(mt + 1) * P],
                                    w2_t[kh][:, nd * 512:(nd + 1) * 512],
                                    start=(kh == 0),
                                    stop=(kh == KHP - 1),
                                )
                            yo = ypool.tile([P, 512], F32, tag="yo")
                            nc.vector.tensor_scalar_mul(yo[:], ys[:], gcol)
                            nc.sync.dma_start(
                                y_d[:, t0 // P + mt, nd * 512:(nd + 1) * 512], yo[:]
                            )
                    t0 += tcs

    nc.finalize()
    return nc


def _install_axon_hooks_shim():
    """The agent image lacks antenv.axon_hooks, but run_bass_kernel_spmd
    imports it unconditionally whenever tracing is requested (including via
    the BASS_TRACE env var). Provide it from trn_agent_boot so tracing is
    safe; degrade silently if neither source exists."""
    import sys
    import types

    try:
        from antenv.axon_hooks import get_axon_ntff_profile_hook  # noqa: F401

        return True
    except ImportError:
        pass
    try:
        from trn_agent_boot.trn_boot import _ntff_profile_via_ctypes

        hook = _ntff_profile_via_ctypes("/opt/axon/libaxon_pjrt.so")
        m = types.ModuleType("antenv.axon_hooks")
        m.get_axon_ntff_profile_hook = lambda: hook
        m.set_axon_ntff_profile_hook = lambda h: None
        sys.modules["antenv.axon_hooks"] = m
        import antenv

        antenv.axon_hooks = m
        return True
    except Exception:
        return False


_HAVE_TRACE_HOOK = _install_axon_hooks_shim()


def _maybe_trace_kwargs():
    lvl = int(os.environ.get("MOE_KERNEL_TRACE", "0"))
    if not lvl or not _HAVE_TRACE_HOOK:
        return {}
    kw = {"trace": True}
    if lvl >= 2:
        kw["trace_cores"] = list(range(E))
    return kw


def kernel(x, Wr, W1, W2):
    global LAST_EXEC_TIME_NS, LAST_RESULTS
    x = np.asarray(x, dtype=np.float32)
    Wr = np.asarray(Wr, dtype=np.float32)
    W1 = np.asarray(W1, dtype=np.float32)
    W2 = np.asarray(W2, dtype=np.float32)

    B, T, Dx = x.shape
    assert Dx == D and Wr.shape == (D, E)
    xf = x.reshape(-1, D)
    N = xf.shape[0]

    # --- routing on host (f64) ---
    logits = xf.astype(np.float64) @ Wr.astype(np.float64)
    m = logits.max(axis=1, keepdims=True)
    p = np.exp(logits - m)
    p /= p.sum(axis=1, keepdims=True)
    ti = np.argpartition(-p, 1, axis=1)[:, :TOPK]          # top-2 experts per token
    tpsum = np.take_along_axis(p, ti, axis=1).sum(axis=1) + EPS

    # load-balance loss on detached stats
    sel = np.zeros((N, E), np.float64)
    np.put_along_axis(sel, ti, 1.0, axis=1)
    imp = p.sum(axis=0)
    load = sel.sum(axis=0)
    imp = imp / (imp.sum() + EPS)
    load = load / (load.sum() + EPS)
    lb = np.float32((imp * load).sum() * E)

    lists = [np.nonzero(sel[:, e])[0] for e in range(E)]
    counts = [len(l) for l in lists]
    maxc = max(counts)
    chunks = _chunks_for(-(-maxc // P))
    C = sum(chunks)
    CT = C // P

    key = tuple(chunks)
    if key not in _NC_CACHE:
        _NC_CACHE[key] = _build(chunks)
    nc = _NC_CACHE[key]

    in_maps = []
    for e in range(E):
        ids = lists[e]
        n_e = len(ids)
        ge = (p[ids, e] / tpsum[ids]).astype(np.float32)

        xt = np.zeros((P, KD, C), np.float32)
        # x_gath.T has shape (D, n_e) with d = kd*128 + p_row
        xt[:, :, :n_e] = xf[ids].T.reshape(KD, P, n_e).transpose(1, 0, 2)

        gfull = np.zeros(C, np.float32)
        gfull[:n_e] = ge
        g_arr = np.ascontiguousarray(gfull.reshape(CT, P).T)

        w1 = np.ascontiguousarray(
            W1[e].reshape(KD, P, KH, P).transpose(2, 1, 0, 3)
        )  # (KH, P, KD, P): [mh, p, kd, hi] = W1[e][kd*128+p, mh*128+hi]
        w2 = np.ascontiguousarray(W2[e].reshape(KH, P, D))

        in_maps.append({"xt": xt, "w1": w1, "w2": w2, "g": g_arr})

    res = run_bass_kernel_spmd(
        nc, in_maps, core_ids=list(range(E)), **_maybe_trace_kwargs()
    )
    LAST_EXEC_TIME_NS = res.exec_time_ns
    LAST_RESULTS = res

    out = np.zeros((N, D), np.float32)
    for e in range(E):
        r = res.results[e]
        y = (np.asarray(r["ya"]) + np.asarray(r["yb"]))     # (P, CT, D)
        y = y.transpose(1, 0, 2).reshape(C, D)[: counts[e]]
        out[lists[e]] += y

    return out.reshape(B, T, D), lb


# revision 20
# speedup vs baseline: 1.0163x; 1.0163x over previous
"""MoE block (top-2 routing, 8 experts) on 8 Trainium2 NeuronCores.

Strategy (expert-parallel, as in the sharding hint):
  - Router (logits -> softmax -> top-2 -> gates) runs on host in f64 as part
    of sharding: the dispatch *is* the shard step, and the gather indices are
    needed on host anyway to build per-core batches and to unshard.
  - Core e owns expert e: it receives the tokens routed to expert e
    (padded to a common capacity C) plus W1[e], W2[e], and computes
    y = gelu(x @ W1) @ W2 * gate with float32r matmuls (full fp32 data,
    ~1 cycle/row on the PE like bf16, ~1e-4 rel err).
  - W1/W2 in fp32 do not fit in SBUF (256 KB/partition), so H=4096 is split
    into two phases of 2048; each phase keeps its W1/W2 halves resident and
    streams token chunks. Each phase writes its partial y (ya / yb).
  - Host combine: out[token] += ya + yb for each expert's token list (each
    token appears in exactly 2 expert lists). Load-balance loss lb is
    computed on host from the routing stats.
"""

import os

import numpy as np

import concourse.bass as bass  # noqa: F401  (engine types come via bacc)
import concourse.mybir as mybir
import concourse.tile as tile
from concourse import bacc
from concourse.bass_utils import run_bass_kernel_spmd

P = 128
D = 1024
H = 4096
E = 8
TOPK = 2
EPS = 1e-9
KD = D // P          # 8 k-tiles over D
KH = H // P          # 32 h-tiles over H
KHP = KH // 2        # 16 h-tiles per phase
F32 = mybir.dt.float32
F32R = mybir.dt.float32r

LAST_EXEC_TIME_NS = None
LAST_RESULTS = None

_NC_CACHE = {}


def _chunks_for(n128):
    """Token-chunk sizes (multiples of 128, each >=256 so float32r matmuls
    run at 1 cycle/row) covering n128*128 tokens."""
    n128 = max(n128, 2)
    full, r = divmod(n128, 4)
    chunks = [512] * full
    if r == 1:
        if full:
            chunks[-1] = 384
            chunks.append(256)
        else:
            chunks = [256]
    elif r == 2:
        chunks.append(256)
    elif r == 3:
        chunks.append(384)
    return chunks


def _build(chunks):
    C = sum(chunks)
    CT = C // P
    nc = bacc.Bacc(None, target_bir_lowering=False)

    xt_d = nc.dram_tensor("xt", [P, KD, C], F32R, kind="ExternalInput")
    w1_d = nc.dram_tensor("w1", [KH, P, KD, P], F32R, kind="ExternalInput")
    w2_d = nc.dram_tensor("w2", [KH, P, D], F32R, kind="ExternalInput")
    g_d = nc.dram_tensor("g", [P, CT], F32, kind="ExternalInput")
    ya_d = nc.dram_tensor("ya", [P, CT, D], F32, kind="ExternalOutput")
    yb_d = nc.dram_tensor("yb", [P, CT, D], F32, kind="ExternalOutput")

    with tile.TileContext(nc) as tc:
        with (
            tc.tile_pool(name="wpool", bufs=1) as wpool,
            tc.tile_pool(name="xpool", bufs=1) as xpool,
            tc.tile_pool(name="hpool", bufs=1) as hpool,
            tc.tile_pool(name="ypool", bufs=4) as ypool,
            tc.tile_pool(name="gpool", bufs=1) as gpool,
            tc.tile_pool(name="ps1", bufs=6, space="PSUM") as ps1,
            tc.tile_pool(name="ps2", bufs=2, space="PSUM") as ps2,
        ):
            g_sb = gpool.tile([P, CT], F32)
            nc.sync.dma_start(g_sb[:], g_d[:])

            for phase, y_d in enumerate((ya_d, yb_d)):
                # chunk-0 x before the 16 MB of weights, split per kd so the
                # first matmul starts after 256 KB instead of 2 MB and the
                # cold-clock ramp hides inside the DMA-paced window
                xt0 = []
                for kd in range(KD):
                    xt0_sub = xpool.tile([P, chunks[0]], F32R, tag=f"xt0_{kd}")
                    xt0.append(xt0_sub)
                w1_t = []
                for mh in range(KHP):
                    t = wpool.tile([P, KD, P], F32R, tag=f"w1_{mh}")
                    w1_t.append(t)
                # queue order: first k-slice of x, first w1 tile (unblocks the
                # first matmul after ~0.75 MB), then the rest in consumption order
                nc.sync.dma_start(xt0[0][:], xt_d[:, 0, 0:chunks[0]])
                nc.sync.dma_start(w1_t[0][:], w1_d[phase * KHP])
                for kd in range(1, KD):
                    nc.sync.dma_start(xt0[kd][:], xt_d[:, kd, 0:chunks[0]])
                for mh in range(1, KHP):
                    nc.sync.dma_start(w1_t[mh][:], w1_d[phase * KHP + mh])
                w2_t = []
                for kh in range(KHP):
                    t = wpool.tile([P, D], F32R, tag=f"w2_{kh}")
                    nc.sync.dma_start(t[:], w2_d[phase * KHP + kh])
                    w2_t.append(t)

                t0 = 0
                for ci, tcs in enumerate(chunks):
                    if ci == 0:
                        xt_k = xt0
                    else:
                        xt = xpool.tile([P, KD, tcs], F32R, tag="xt")
                        nc.sync.dma_start(xt[:], xt_d[:, :, t0:t0 + tcs])
                        xt_k = [xt[:, kd, :] for kd in range(KD)]

                    # h[mh] = gelu(W1_tile.T @ x_chunk), laid out (H-part, token)
                    h_t = []
                    for mh in range(KHP):
                        ps = ps1.tile([P, tcs], F32, tag="ps")
                        for kd in range(KD):
                            nc.tensor.matmul(
                                ps[:],
                                w1_t[mh][:, kd, :],
                                xt_k[kd][:] if ci == 0 else xt_k[kd],
                                start=(kd == 0),
                                stop=(kd == KD - 1),
                            )
                        hb = hpool.tile([P, tcs], F32R, tag=f"h{mh}")
                        nc.scalar.activation(
                            hb[:], ps[:], mybir.ActivationFunctionType.Gelu
                        )
                        h_t.append(hb)

                    # y[token-tile, d] = h.T @ W2, scaled by per-token gate
                    for mt in range(tcs // P):
                        gcol = g_sb[:, t0 // P + mt: t0 // P + mt + 1]
                        for nd in range(2):
                            ys = ps2.tile([P, 512], F32, tag="ys")
                            for kh in range(KHP):
                                nc.tensor.matmul(
                                    ys[:],
                                    h_t[kh][:, mt * P:(mt + 1) * P],
                                    w2_t[kh][:, nd * 512:(nd + 1) * 512],
                                    start=(kh == 0),
                                    stop=(kh == KHP - 1),
                                )
                            yo = ypool.tile([P, 512], F32, tag="yo")
                            nc.vector.tensor_scalar_mul(yo[:], ys[:], gcol)
                            nc.sync.dma_start(
                                y_d[:, t0 // P + mt, nd * 512:(nd + 1) * 512], yo[:]
                            )
                    t0 += tcs

    nc.finalize()
    return nc


def _install_axon_hooks_shim():
    """The agent image lacks antenv.axon_hooks, but run_bass_kernel_spmd
    imports it unconditionally whenever tracing is requested (including via
    the BASS_TRACE env var). Provide it from trn_agent_boot so tracing is
    safe; degrade silently if neither source exists."""
    import sys
    import types

    try:
        from antenv.axon_hooks import get_axon_ntff_profile_hook  # noqa: F401

        return True
    except ImportError:
        pass
    try:
        from trn_agent_boot.trn_boot import _ntff_profile_via_ctypes

        hook = _ntff_profile_via_ctypes("/opt/axon/libaxon_pjrt.so")
        m = types.ModuleType("antenv.axon_hooks")
        m.get_axon_ntff_profile_hook = lambda: hook
        m.set_axon_ntff_profile_hook = lambda h: None
        sys.modules["antenv.axon_hooks"] = m
        import antenv

        antenv.axon_hooks = m
        return True
    except Exception:
        return False


_HAVE_TRACE_HOOK = _install_axon_hooks_shim()


def _maybe_trace_kwargs():
    lvl = int(os.environ.get("MOE_KERNEL_TRACE", "0"))
    if not lvl or not _HAVE_TRACE_HOOK:
        return {}
    kw = {"trace": True}
    if lvl >= 2:
        kw["trace_cores"] = list(range(E))
    return kw


def kernel(x, Wr, W1, W2):
    global LAST_EXEC_TIME_NS, LAST_RESULTS
    x = np.asarray(x, dtype=np.float32)
    Wr = np.asarray(Wr, dtype=np.float32)
    W1 = np.asarray(W1, dtype=np.float32)
    W2 = np.asarray(W2, dtype=np.float32)

    B, T, Dx = x.shape
    assert Dx == D and Wr.shape == (D, E)
    xf = x.reshape(-1, D)
    N = xf.shape[0]

    # --- routing on host (f64) ---
    logits = xf.astype(np.float64) @ Wr.astype(np.float64)
    m = logits.max(axis=1, keepdims=True)
    p = np.exp(logits - m)
    p /= p.sum(axis=1, keepdims=True)
    ti = np.argpartition(-p, 1, axis=1)[:, :TOPK]          # top-2 experts per token
    tpsum = np.take_along_axis(p, ti, axis=1).sum(axis=1) + EPS

    # load-balance loss on detached stats
    sel = np.zeros((N, E), np.float64)
    np.put_along_axis(sel, ti, 1.0, axis=1)
    imp = p.sum(axis=0)
    load = sel.sum(axis=0)
    imp = imp / (imp.sum() + EPS)
    load = load / (load.sum() + EPS)
    lb = np.float32((imp * load).sum() * E)

    lists = [np.nonzero(sel[:, e])[0] for e in range(E)]
    counts = [len(l) for l in lists]
    maxc = max(counts)
    chunks = _chunks_for(-(-maxc // P))
    C = sum(chunks)
    CT = C // P

    key = tuple(chunks)
    if key not in _NC_CACHE:
        _NC_CACHE[key] = _build(chunks)
    nc = _NC_CACHE[key]

    in_maps = []
    for e in range(E):
        ids = lists[e]
        n_e = len(ids)
        ge = (p[ids, e] / tpsum[ids]).astype(np.float32)

        xt = np.zeros((P, KD, C), np.float32)
        # x_gath.T has shape (D, n_e) with d = kd*128 + p_row
        xt[:, :, :n_e] = xf[ids].T.reshape(KD, P, n_e).transpose(1, 0, 2)

        gfull = np.zeros(C, np.float32)
        gfull[:n_e] = ge
        g_arr = np.ascontiguousarray(gfull.reshape(CT, P).T)

        w1 = np.ascontiguousarray(
            W1[e].reshape(KD, P, KH, P).transpose(2, 1, 0, 3)
        )  # (KH, P, KD, P): [mh, p, kd, hi] = W1[e][kd*128+p, mh*128+hi]
        w2 = np.ascontiguousarray(W2[e].reshape(KH, P, D))

        in_maps.append({"xt": xt, "w1": w1, "w2": w2, "g": g_arr})

    res = run_bass_kernel_spmd(
        nc, in_maps, core_ids=list(range(E)), **_maybe_trace_kwargs()
    )
    LAST_EXEC_TIME_NS = res.exec_time_ns
    LAST_RESULTS = res

    out = np.zeros((N, D), np.float32)
    for e in range(E):
        r = res.results[e]
        y = (np.asarray(r["ya"]) + np.asarray(r["yb"]))     # (P, CT, D)
        y = y.transpose(1, 0, 2).reshape(C, D)[: counts[e]]
        out[lists[e]] += y

    return out.reshape(B, T, D), lb


# revision 21
# speedup vs baseline: 1.0299x; 1.0134x over previous
"""MoE block (top-2 routing, 8 experts) on 8 Trainium2 NeuronCores.

Strategy (expert-parallel, as in the sharding hint):
  - Router (logits -> softmax -> top-2 -> gates) runs on host in f64 as part
    of sharding: the dispatch *is* the shard step, and the gather indices are
    needed on host anyway to build per-core batches and to unshard.
  - Core e owns expert e: it receives the tokens routed to expert e
    (padded to a common capacity C) plus W1[e], W2[e], and computes
    y = gelu(x @ W1) @ W2 * gate with float32r matmuls (full fp32 data,
    ~1 cycle/row on the PE like bf16, ~1e-4 rel err).
  - W1/W2 in fp32 do not fit in SBUF (256 KB/partition), so H=4096 is split
    into two phases of 2048; each phase keeps its W1/W2 halves resident and
    streams token chunks. Each phase writes its partial y (ya / yb).
  - Host combine: out[token] += ya + yb for each expert's token list (each
    token appears in exactly 2 expert lists). Load-balance loss lb is
    computed on host from the routing stats.
"""

import os

import numpy as np

import concourse.bass as bass  # noqa: F401  (engine types come via bacc)
import concourse.mybir as mybir
import concourse.tile as tile
from concourse import bacc
from concourse.bass_utils import run_bass_kernel_spmd

P = 128
D = 1024
H = 4096
E = 8
TOPK = 2
EPS = 1e-9
KD = D // P          # 8 k-tiles over D
KH = H // P          # 32 h-tiles over H
KHP = KH // 2        # 16 h-tiles per phase
F32 = mybir.dt.float32
F32R = mybir.dt.float32r

LAST_EXEC_TIME_NS = None
LAST_RESULTS = None

_NC_CACHE = {}


def _chunks_for(n128):
    """Token-chunk sizes (multiples of 128, each >=256 so float32r matmuls
    run at 1 cycle/row) covering n128*128 tokens."""
    n128 = max(n128, 2)
    full, r = divmod(n128, 4)
    chunks = [512] * full
    if r == 1:
        if full:
            chunks[-1] = 384
            chunks.append(256)
        else:
            chunks = [256]
    elif r == 2:
        chunks.append(256)
    elif r == 3:
        chunks.append(384)
    return chunks


def _build(chunks):
    C = sum(chunks)
    CT = C // P
    nc = bacc.Bacc(None, target_bir_lowering=False)

    xt_d = nc.dram_tensor("xt", [P, KD, C], F32R, kind="ExternalInput")
    w1_d = nc.dram_tensor("w1", [KH, P, KD, P], F32R, kind="ExternalInput")
    w2_d = nc.dram_tensor("w2", [KH, P, D], F32R, kind="ExternalInput")
    g_d = nc.dram_tensor("g", [P, CT], F32, kind="ExternalInput")
    ya_d = nc.dram_tensor("ya", [P, CT, D], F32, kind="ExternalOutput")
    yb_d = nc.dram_tensor("yb", [P, CT, D], F32, kind="ExternalOutput")

    with tile.TileContext(nc) as tc:
        with (
            tc.tile_pool(name="wpool", bufs=1) as wpool,
            tc.tile_pool(name="xpool", bufs=1) as xpool,
            tc.tile_pool(name="hpool", bufs=1) as hpool,
            tc.tile_pool(name="ypool", bufs=4) as ypool,
            tc.tile_pool(name="gpool", bufs=1) as gpool,
            tc.tile_pool(name="ps1", bufs=6, space="PSUM") as ps1,
            tc.tile_pool(name="ps2", bufs=2, space="PSUM") as ps2,
        ):
            g_sb = gpool.tile([P, CT], F32)
            nc.sync.dma_start(g_sb[:], g_d[:])

            for phase, y_d in enumerate((ya_d, yb_d)):
                # chunk-0 x before the 16 MB of weights, split per kd so the
                # first matmul starts after 256 KB instead of 2 MB and the
                # cold-clock ramp hides inside the DMA-paced window
                xt0 = []
                for kd in range(KD):
                    xt0_sub = xpool.tile([P, chunks[0]], F32R, tag=f"xt0_{kd}")
                    xt0.append(xt0_sub)
                w1_t = []
                for mh in range(KHP):
                    t = wpool.tile([P, KD, P], F32R, tag=f"w1_{mh}")
                    w1_t.append(t)
                # queue order: first k-slice of x, first w1 tile (unblocks the
                # first matmul after ~0.75 MB), then the rest in consumption order
                nc.sync.dma_start(xt0[0][:], xt_d[:, 0, 0:chunks[0]])
                nc.sync.dma_start(w1_t[0][:], w1_d[phase * KHP])
                for kd in range(1, KD):
                    nc.sync.dma_start(xt0[kd][:], xt_d[:, kd, 0:chunks[0]])
                for mh in range(1, KHP):
                    nc.sync.dma_start(w1_t[mh][:], w1_d[phase * KHP + mh])
                # w2 split by output half, nd0 tiles queued first: chunk-0's
                # first mm2 groups (nd=0) need only 4 MB of w2, not 8
                w2_t = [[None] * 2 for _ in range(KHP)]
                for nd in range(2):
                    for kh in range(KHP):
                        w2_sub = wpool.tile([P, 512], F32R, tag=f"w2_{kh}_{nd}")
                        nc.sync.dma_start(
                            w2_sub[:],
                            w2_d[phase * KHP + kh, :, nd * 512:(nd + 1) * 512],
                        )
                        w2_t[kh][nd] = w2_sub

                t0 = 0
                for ci, tcs in enumerate(chunks):
                    if ci == 0:
                        xt_k = xt0
                    else:
                        xt = xpool.tile([P, KD, tcs], F32R, tag="xt")
                        nc.sync.dma_start(xt[:], xt_d[:, :, t0:t0 + tcs])
                        xt_k = [xt[:, kd, :] for kd in range(KD)]

                    # h[mh] = gelu(W1_tile.T @ x_chunk), laid out (H-part, token)
                    h_t = []
                    for mh in range(KHP):
                        ps = ps1.tile([P, tcs], F32, tag="ps")
                        for kd in range(KD):
                            nc.tensor.matmul(
                                ps[:],
                                w1_t[mh][:, kd, :],
                                xt_k[kd][:] if ci == 0 else xt_k[kd],
                                start=(kd == 0),
                                stop=(kd == KD - 1),
                            )
                        hb = hpool.tile([P, tcs], F32R, tag=f"h{mh}")
                        nc.scalar.activation(
                            hb[:], ps[:], mybir.ActivationFunctionType.Gelu
                        )
                        h_t.append(hb)

                    # y[token-tile, d] = h.T @ W2, scaled by per-token gate
                    # (nd outer so chunk-0 needs only the nd0 w2 halves first)
                    for nd in range(2):
                        for mt in range(tcs // P):
                            gcol = g_sb[:, t0 // P + mt: t0 // P + mt + 1]
                            ys = ps2.tile([P, 512], F32, tag="ys")
                            for kh in range(KHP):
                                nc.tensor.matmul(
                                    ys[:],
                                    h_t[kh][:, mt * P:(mt + 1) * P],
                                    w2_t[kh][nd][:],
                                    start=(kh == 0),
                                    stop=(kh == KHP - 1),
                                )
                            yo = ypool.tile([P, 512], F32, tag="yo")
                            nc.vector.tensor_scalar_mul(yo[:], ys[:], gcol)
                            nc.sync.dma_start(
                                y_d[:, t0 // P + mt, nd * 512:(nd + 1) * 512], yo[:]
                            )
                    t0 += tcs

    nc.finalize()
    return nc


def _install_axon_hooks_shim():
    """The agent image lacks antenv.axon_hooks, but run_bass_kernel_spmd
    imports it unconditionally whenever tracing is requested (including via
    the BASS_TRACE env var). Provide it from trn_agent_boot so tracing is
    safe; degrade silently if neither source exists."""
    import sys
    import types

    try:
        from antenv.axon_hooks import get_axon_ntff_profile_hook  # noqa: F401

        return True
    except ImportError:
        pass
    try:
        from trn_agent_boot.trn_boot import _ntff_profile_via_ctypes

        hook = _ntff_profile_via_ctypes("/opt/axon/libaxon_pjrt.so")
        m = types.ModuleType("antenv.axon_hooks")
        m.get_axon_ntff_profile_hook = lambda: hook
        m.set_axon_ntff_profile_hook = lambda h: None
        sys.modules["antenv.axon_hooks"] = m
        import antenv

        antenv.axon_hooks = m
        return True
    except Exception:
        return False


_HAVE_TRACE_HOOK = _install_axon_hooks_shim()


def _maybe_trace_kwargs():
    lvl = int(os.environ.get("MOE_KERNEL_TRACE", "0"))
    if not lvl or not _HAVE_TRACE_HOOK:
        return {}
    kw = {"trace": True}
    if lvl >= 2:
        kw["trace_cores"] = list(range(E))
    return kw


def kernel(x, Wr, W1, W2):
    global LAST_EXEC_TIME_NS, LAST_RESULTS
    x = np.asarray(x, dtype=np.float32)
    Wr = np.asarray(Wr, dtype=np.float32)
    W1 = np.asarray(W1, dtype=np.float32)
    W2 = np.asarray(W2, dtype=np.float32)

    B, T, Dx = x.shape
    assert Dx == D and Wr.shape == (D, E)
    xf = x.reshape(-1, D)
    N = xf.shape[0]

    # --- routing on host (f64) ---
    logits = xf.astype(np.float64) @ Wr.astype(np.float64)
    m = logits.max(axis=1, keepdims=True)
    p = np.exp(logits - m)
    p /= p.sum(axis=1, keepdims=True)
    ti = np.argpartition(-p, 1, axis=1)[:, :TOPK]          # top-2 experts per token
    tpsum = np.take_along_axis(p, ti, axis=1).sum(axis=1) + EPS

    # load-balance loss on detached stats
    sel = np.zeros((N, E), np.float64)
    np.put_along_axis(sel, ti, 1.0, axis=1)
    imp = p.sum(axis=0)
    load = sel.sum(axis=0)
    imp = imp / (imp.sum() + EPS)
    load = load / (load.sum() + EPS)
    lb = np.float32((imp * load).sum() * E)

    lists = [np.nonzero(sel[:, e])[0] for e in range(E)]
    counts = [len(l) for l in lists]
    maxc = max(counts)
    chunks = _chunks_for(-(-maxc // P))
    C = sum(chunks)
    CT = C // P

    key = tuple(chunks)
    if key not in _NC_CACHE:
        _NC_CACHE[key] = _build(chunks)
    nc = _NC_CACHE[key]

    in_maps = []
    for e in range(E):
        ids = lists[e]
        n_e = len(ids)
        ge = (p[ids, e] / tpsum[ids]).astype(np.float32)

        xt = np.zeros((P, KD, C), np.float32)
        # x_gath.T has shape (D, n_e) with d = kd*128 + p_row
        xt[:, :, :n_e] = xf[ids].T.reshape(KD, P, n_e).transpose(1, 0, 2)

        gfull = np.zeros(C, np.float32)
        gfull[:n_e] = ge
        g_arr = np.ascontiguousarray(gfull.reshape(CT, P).T)

        w1 = np.ascontiguousarray(
            W1[e].reshape(KD, P, KH, P).transpose(2, 1, 0, 3)
        )  # (KH, P, KD, P): [mh, p, kd, hi] = W1[e][kd*128+p, mh*128+hi]
        w2 = np.ascontiguousarray(W2[e].reshape(KH, P, D))

        in_maps.append({"xt": xt, "w1": w1, "w2": w2, "g": g_arr})

    res = run_bass_kernel_spmd(
        nc, in_maps, core_ids=list(range(E)), **_maybe_trace_kwargs()
    )
    LAST_EXEC_TIME_NS = res.exec_time_ns
    LAST_RESULTS = res

    out = np.zeros((N, D), np.float32)
    for e in range(E):
        r = res.results[e]
        y = (np.asarray(r["ya"]) + np.asarray(r["yb"]))     # (P, CT, D)
        y = y.transpose(1, 0, 2).reshape(C, D)[: counts[e]]
        out[lists[e]] += y

    return out.reshape(B, T, D), lb
